# revision 9
# baseline (speedup 1.0000x reference)
"""Trainium2 Bass kernel for nn_ContrastiveLearning (self-contained).

kernel(**inputs) takes the FULL unsharded inputs (as produced by the
problem's setup_inputs) and returns (logits_per_img, logits_per_depth),
each [4, 100, 100] fp32.

Sharding: 8 NeuronCores, core c = (batch b=c//2, modality m=c%2). Each core
streams its 26 MB feature map in 10 one-patch-row slabs (2.62 MB each,
alternating between the sync and gpsimd DMA queues), computes
conv1x1+ReLU per slab, then runs the patch MLP + LayerNorm per slab-PAIR
(5 chains of 20 patches, interleaved with the conv stream so only the last
chain sits in the tail), exchanges eT with its pair partner (2-core
AllGather), and computes the 100x100 contrastive logits on-device.
sqrt(exp(logit_scale)) is folded into the LayerNorm affine on the host, so
no logits scaling happens on device.
"""
import numpy as np
import concourse.bass as bass
import concourse.bacc as bacc
import concourse.mybir as mybir
import concourse.tile as tile
from concourse.bass_utils import run_bass_kernel_spmd


F32 = mybir.dt.float32
F32R = mybir.dt.float32r
AF = mybir.ActivationFunctionType
ALU = mybir.AluOpType

NV = NH = 10          # patch grid
NP = NV * NH          # 100 patches
CPS = 16
ENC = 128
PIX = CPS * CPS       # 256 features per patch
LN_EPS = 1e-5
N_SLABS = 10          # one patch-row (16 rows x 160 cols) per slab
SLAB_NP = NP // N_SLABS   # 10 patches per slab
CH_NP = 2 * SLAB_NP       # 20 patches per chain (slab pair)


def build_kernel(nc, exchange='cc', n_cores=8, conv_f32r=True):
    cdt = F32R if conv_f32r else F32

    feat = nc.dram_tensor("feat", [2, 128, 160, 160], cdt, kind="ExternalInput")
    convw = nc.dram_tensor("convw", [128, 2], cdt, kind="ExternalInput")
    bias128 = nc.dram_tensor("bias128", [128, 1], F32, kind="ExternalInput")
    w1t = nc.dram_tensor("w1t", [128, 512], F32, kind="ExternalInput")
    w2t = nc.dram_tensor("w2t", [128, 256], F32, kind="ExternalInput")
    ln_g = nc.dram_tensor("ln_g", [128, 1], F32, kind="ExternalInput")
    ln_b = nc.dram_tensor("ln_b", [128, 1], F32, kind="ExternalInput")
    ident = nc.dram_tensor("ident", [128, 128], F32, kind="ExternalInput")
    logits = nc.dram_tensor("logits", [NP, NP], F32, kind="ExternalOutput")

    if exchange == 'cc':
        cc_in = nc.dram_tensor("cc_in", [ENC, NP], F32)
        cc_out = nc.dram_tensor("cc_out", [2 * ENC, NP], F32)

    with tile.TileContext(nc) as tc:
        with (
            tc.tile_pool(name="slab", bufs=5) as slab_pool,
            tc.tile_pool(name="x1p", bufs=2) as x1_pool,
            tc.tile_pool(name="cst", bufs=1) as cst,
            tc.tile_pool(name="work", bufs=1) as work,
            tc.tile_pool(name="rot", bufs=2) as rot,
            tc.tile_pool(name="cv", bufs=3, space="PSUM") as ps_cv,
            tc.tile_pool(name="mm", bufs=4, space="PSUM") as ps_mm,
            tc.tile_pool(name="lg", bufs=1, space="PSUM") as ps_lg,
        ):
            # constants / weights on the scalar (ACT) queue
            convw_s = cst.tile([128, 2], cdt, tag="convw")
            bias_s = cst.tile([128, 1], F32, tag="bias")
            w1t_s = cst.tile([128, 512], F32, tag="w1t")
            w2t_s = cst.tile([128, 256], F32, tag="w2t")
            g_s = cst.tile([128, 1], F32, tag="g")
            b_s = cst.tile([128, 1], F32, tag="b")
            id_s = cst.tile([128, 128], F32, tag="id")
            ones_col = cst.tile([128, 1], F32, tag="onec")
            ones_row = cst.tile([1, 128], F32, tag="oner")
            for t, srct in ((convw_s, convw), (bias_s, bias128), (w1t_s, w1t),
                            (w2t_s, w2t), (g_s, ln_g), (b_s, ln_b),
                            (id_s, ident)):
                nc.scalar.dma_start(t[:], srct[:])
            nc.gpsimd.memset(ones_col[:], 1.0)
            nc.gpsimd.memset(ones_row[:], 1.0)

            # warm the ACT function tables (Relu/Sqrt) before the stream needs them
            scr = work.tile([1, 2], F32, tag="scr")
            nc.gpsimd.memset(scr[:], 1.0)
            nc.scalar.activation(scr[0:1, 0:1], scr[0:1, 1:2], AF.Sqrt)
            nc.scalar.activation(scr[0:1, 0:1], scr[0:1, 1:2], AF.Relu)

            # persistent tiles
            xT = [work.tile([128, NP], F32, tag=f"xT{v}", name=f"xT{v}")
                  for v in range(2)]
            hT = [work.tile([128, NP], F32, tag=f"hT{t}", name=f"hT{t}")
                  for t in range(2)]
            yT = work.tile([128, NP], F32, tag="yT")
            eT = work.tile([128, NP], F32, tag="eT")
            B_s = work.tile([128, NP], F32, tag="B")

            # all slab DMAs up front, alternating sync/gpsimd queues
            slabs = []
            for s in range(N_SLABS):
                st = slab_pool.tile([128, 2, CPS, NH * CPS], cdt, tag="slab")
                eng = nc.sync if s % 2 == 0 else nc.gpsimd
                eng.dma_start(st[:], feat[:, :, s * CPS:(s + 1) * CPS, :].rearrange(
                    "u p h w -> p u h w"))
                slabs.append(st)

            x1s = {}

            def conv_slab(s):
                """conv1x1 + ReLU for slab s into x1 pair staging (10 patches)."""
                k = s // 2
                if s % 2 == 0:
                    x1s[k] = x1_pool.tile([1, CH_NP, PIX], F32, tag="x1",
                                          name=f"x1_{k}")
                x1 = x1s[k]
                base = (s % 2) * SLAB_NP
                stv = [slabs[s][:, u, :, :].rearrange("p h (c j) -> p c h j", c=NH)
                       for u in range(2)]
                for g in range(SLAB_NP // 2):
                    cvt = ps_cv.tile([1, 2 * PIX], F32, tag="cv")
                    for u in range(2):
                        nc.tensor.matmul(
                            cvt[0:1, :],
                            convw_s[:, u:u + 1],
                            stv[u][:, 2 * g:2 * g + 2, :, :],
                            start=(u == 0), stop=(u == 1),
                        )
                    # evacuate + bias + relu; ACT takes even groups, DVE odd
                    o = x1[0:1, base + 2 * g:base + 2 * g + 2, :]
                    if g % 2 == 0:
                        nc.scalar.activation(o, cvt[0:1, :], AF.Relu,
                                             bias=bias_s[0:1, :])
                    else:
                        nc.vector.tensor_scalar(o, cvt[0:1, :], bias_s[0:1, :],
                                                0.0, ALU.add, ALU.max)

            chst = {}

            def chain_a(k):
                """pair k: compact + transpose + MLP into yT[:, J] (20 patches)."""
                n = CH_NP
                J = slice(n * k, n * (k + 1))
                x_cmp = rot.tile([n, PIX], F32, tag="xc", name=f"xc_{k}")
                nc.scalar.dma_start(x_cmp[:, :], x1s[k][0:1, :, :])
                for v in range(2):
                    tp = ps_mm.tile([128, n], F32, tag="mm", name=f"tp{v}_{k}")
                    nc.tensor.transpose(tp[:], x_cmp[:, 128 * v:128 * (v + 1)],
                                        id_s[0:n, 0:n])
                    if v == 0:
                        nc.scalar.copy(xT[v][:, J], tp[:])
                    else:
                        nc.vector.tensor_copy(xT[v][:, J], tp[:])
                for t in range(2):
                    ph = ps_mm.tile([128, n], F32, tag="mm", name=f"ph{t}_{k}")
                    for v in range(2):
                        nc.tensor.matmul(
                            ph[:], w1t_s[:, 256 * v + 128 * t:256 * v + 128 * t + 128],
                            xT[v][:, J], start=(v == 0), stop=(v == 1),
                        )
                    nc.scalar.activation(hT[t][:, J], ph[:], AF.Relu)
                py = ps_mm.tile([128, n], F32, tag="mm", name=f"py_{k}")
                for t in range(2):
                    nc.tensor.matmul(py[:], w2t_s[:, 128 * t:128 * t + 128],
                                     hT[t][:, J], start=(t == 0), stop=(t == 1))
                nc.vector.tensor_copy(yT[:, J], py[:])
                # stats: sum(y) and sum(y^2) via ones-matmuls into one bank
                sq = rot.tile([128, n], F32, tag="sq", name=f"sq_{k}")
                nc.vector.tensor_tensor(sq[:], yT[:, J], yT[:, J], ALU.mult)
                row = ps_mm.tile([1, 64], F32, tag="mm", name=f"row_{k}")
                nc.tensor.matmul(row[0:1, 0:n], ones_col[:], yT[:, J],
                                 start=True, stop=True)
                nc.tensor.matmul(row[0:1, 32:32 + n], ones_col[:], sq[:],
                                 start=True, stop=True)
                chst[k] = row

            def chain_b(k):
                """pair k: LN row math + broadcast + affine into eT[:, J]."""
                n = CH_NP
                J = slice(n * k, n * (k + 1))
                row = chst.pop(k)
                rw = rot.tile([1, 128], F32, tag="rw", name=f"rw_{k}")
                mrow, qrow = rw[0:1, 0:n], rw[0:1, 20:20 + n]
                t1, veps = rw[0:1, 40:40 + n], rw[0:1, 60:60 + n]
                srow = rw[0:1, 80:80 + n]
                nc.vector.tensor_scalar_mul(mrow, row[0:1, 0:n], 1.0 / ENC)
                nc.vector.tensor_scalar_mul(qrow, row[0:1, 32:32 + n], 1.0 / ENC)
                nc.vector.tensor_tensor(t1, mrow, mrow, ALU.mult)
                nc.vector.tensor_tensor(veps, qrow, t1, ALU.subtract)
                nc.vector.tensor_scalar_add(veps, veps, LN_EPS)
                nc.scalar.activation(srow, veps, AF.Sqrt)
                # rstd into qrow slot; nmr = -mean*rstd into mrow slot
                rstd = qrow
                nc.vector.reciprocal(rstd, srow)
                # one Newton polish step: r' = r * (1.5 - 0.5*v*r^2)
                nc.vector.tensor_tensor(t1, rstd, rstd, ALU.mult)
                nc.vector.tensor_tensor(t1, t1, veps, ALU.mult)
                nc.vector.tensor_scalar(t1, t1, -0.5, 1.5, ALU.mult, ALU.add)
                nc.vector.tensor_tensor(rstd, rstd, t1, ALU.mult)
                nc.vector.tensor_tensor(mrow, mrow, rstd, ALU.mult)
                nc.vector.tensor_scalar_mul(mrow, mrow, -1.0)  # nmr
                bc = ps_mm.tile([128, 64], F32, tag="mm", name=f"bc_{k}")
                nc.tensor.matmul(bc[:, 0:n], ones_row[:], rstd, start=True, stop=True)
                nc.tensor.matmul(bc[:, 32:32 + n], ones_row[:], mrow,
                                 start=True, stop=True)
                nc.vector.tensor_tensor(yT[:, J], yT[:, J], bc[:, 0:n], ALU.mult)
                nc.vector.tensor_tensor(yT[:, J], yT[:, J], bc[:, 32:32 + n], ALU.add)
                # affine (ln_g/ln_b pre-scaled by sqrt(exp(logit_scale)) on host)
                nc.vector.tensor_scalar(eT[:, J], yT[:, J], g_s[:], b_s[:],
                                        ALU.mult, ALU.add)

            # interleave: convs stream in, chains lag by one slab to avoid
            # stalling the PE FIFO on vector-engine dependencies
            conv_slab(0); conv_slab(1); conv_slab(2)
            chain_a(0)
            conv_slab(3)
            chain_b(0)
            conv_slab(4)
            chain_a(1)
            conv_slab(5)
            chain_b(1)
            conv_slab(6)
            chain_a(2)
            conv_slab(7)
            chain_b(2)
            conv_slab(8)
            chain_a(3)
            conv_slab(9)
            chain_b(3)
            chain_a(4)
            chain_b(4)

            # exchange eT within pairs
            if exchange == 'rdma':
                rsem = nc.alloc_semaphore("rdma_rsem")
                lsem = nc.alloc_semaphore("rdma_lsem")
                nc.gpsimd.remote_dma_broadcast(
                    B_s[:], eT[:], remote_sem=rsem, local_sem=lsem,
                    rdests=[(0, 1)] + [None] * 7)
                nc.gpsimd.trigger_dma(count=None)
                with tc.tile_critical():
                    nc.vector.wait_ge(rsem, 2)
                    nc.vector.tensor_copy(B_s[:], B_s[:])
            elif exchange == 'cc':
                nc.scalar.dma_start(cc_in[:], eT[:])
                groups = [[2 * i, 2 * i + 1] for i in range(n_cores // 2)]
                nc.gpsimd.collective_compute(
                    "AllGather", ALU.bypass, replica_groups=groups,
                    ins=[cc_in.ap().opt()], outs=[cc_out.ap().opt()],
                )
                # even cores read the odd partner's shard (only even cores'
                # logits are collected on the host)
                nc.scalar.dma_start(B_s[:], cc_out[ENC:2 * ENC, :])
            else:
                nc.vector.tensor_copy(B_s[:], eT[:])

            # logits = eT.T @ B  (logit scale folded into eT/B via ln_g/ln_b)
            pL = ps_lg.tile([NP, NP], F32, tag="lg")
            nc.tensor.matmul(pL[:], eT[:], B_s[:], start=True, stop=True)
            L_s = work.tile([NP, NP], F32, tag="Ls")
            nc.vector.tensor_copy(L_s[:], pL[:])
            nc.sync.dma_start(logits[:], L_s[:])

    nc.compile()
    return nc


def host_inputs_for_core(core, inputs):
    """Build the per-core in_map from the full problem inputs dict."""
    b, m = core // 2, core % 2
    feat = np.asarray(inputs["feat_c1" if m == 0 else "feat_c2"])[b]
    pre = "img_" if m == 0 else "depth_"
    s_half = np.sqrt(np.exp(np.float32(np.asarray(inputs["logit_scale"]))))
    cw = np.asarray(inputs[pre + "conv_w"]).reshape(2, 128).T.copy()   # [128,2]
    cb = np.full((128, 1), np.asarray(inputs[pre + "conv_b"])[0], np.float32)
    w1 = np.asarray(inputs[pre + "w1"])  # [256,256] (o=128t+m', i=128v+k)
    w1t = np.ascontiguousarray(
        w1.reshape(2, 128, 2, 128).transpose(3, 2, 0, 1).reshape(128, 512))
    w2 = np.asarray(inputs[pre + "w2"])  # [128,256]
    w2t = np.ascontiguousarray(
        w2.reshape(128, 2, 128).transpose(2, 1, 0).reshape(128, 256))
    return {
        "feat": np.ascontiguousarray(feat).reshape(2, 128, 160, 160),
        "convw": cw.astype(np.float32),
        "bias128": cb,
        "w1t": w1t.astype(np.float32),
        "w2t": w2t.astype(np.float32),
        "ln_g": (np.asarray(inputs[pre + "ln_g"]) * s_half)
            .reshape(128, 1).astype(np.float32),
        "ln_b": (np.asarray(inputs[pre + "ln_b"]) * s_half)
            .reshape(128, 1).astype(np.float32),
        "ident": np.eye(128, dtype=np.float32),
    }


_NC_CACHE = {}


def _get_nc():
    if "nc" not in _NC_CACHE:
        import os
        nc = bacc.Bacc("TRN2", target_bir_lowering=False, num_devices=8)
        build_kernel(nc, exchange=os.environ.get("BASS_EXCHANGE", "cc"))
        _NC_CACHE["nc"] = nc
    return _NC_CACHE["nc"]


def kernel(**inputs):
    nc = _get_nc()
    in_maps = [host_inputs_for_core(c, inputs) for c in range(8)]
    res = run_bass_kernel_spmd(nc, in_maps, list(range(8)))
    logits_img = np.stack([np.asarray(res.results[2 * b]["logits"])
                           for b in range(4)]).astype(np.float32)
    logits_depth = np.ascontiguousarray(logits_img.transpose(0, 2, 1))
    return logits_img, logits_depth


# revision 41
# speedup vs baseline: 1.0019x; 1.0019x over previous
"""Trainium2 Bass kernel for nn_ContrastiveLearning (self-contained).

kernel(**inputs) takes the FULL unsharded inputs (as produced by the
problem's setup_inputs) and returns (logits_per_img, logits_per_depth),
each [4, 100, 100] fp32.

Sharding: 8 NeuronCores, core c = (batch b=c//2, modality m=c%2). Each core
streams its 26 MB feature map in 10 one-patch-row slabs (2.62 MB each,
alternating between the sync and gpsimd DMA queues), computes
conv1x1+ReLU per slab, then runs the patch MLP + LayerNorm per slab-PAIR
(5 chains of 20 patches, interleaved with the conv stream so only the last
chain sits in the tail), exchanges eT with its pair partner (2-core
AllGather), and computes the 100x100 contrastive logits on-device.
sqrt(exp(logit_scale)) is folded into the LayerNorm affine on the host, so
no logits scaling happens on device.
"""
import numpy as np
import concourse.bass as bass
import concourse.bacc as bacc
import concourse.mybir as mybir
import concourse.tile as tile
from concourse.bass_utils import run_bass_kernel_spmd


F32 = mybir.dt.float32
F32R = mybir.dt.float32r
AF = mybir.ActivationFunctionType
ALU = mybir.AluOpType

NV = NH = 10          # patch grid
NP = NV * NH          # 100 patches
CPS = 16
ENC = 128
PIX = CPS * CPS       # 256 features per patch
LN_EPS = 1e-5
N_SLABS = 10          # one patch-row (16 rows x 160 cols) per slab
SLAB_NP = NP // N_SLABS   # 10 patches per slab
CH_NP = 2 * SLAB_NP       # 20 patches per chain (slab pair)


BF16 = mybir.dt.bfloat16

# chains: last two slabs get their own chain so only ~10 patches of MLP/LN
# work remains after the final slab DMA lands
CHAINS = [(0, 0, 20), (1, 20, 20), (2, 40, 20), (3, 60, 20),
          (4, 80, 10), (5, 90, 10)]
SLAB_CHAIN = {0: (0, 0), 1: (0, 10), 2: (1, 0), 3: (1, 10),
              4: (2, 0), 5: (2, 10), 6: (3, 0), 7: (3, 10),
              8: (4, 0), 9: (5, 0)}


# NOTE: bf16 exchange payload (xdt=BF16) corrupts data through the real
# AllGather on this stack — keep the exchange in f32
def build_kernel(nc, exchange='cc', n_cores=8, conv_f32r=True, xdt=F32):
    cdt = F32R if conv_f32r else F32

    feat = nc.dram_tensor("feat", [2, 128, 160, 160], cdt, kind="ExternalInput")
    convw = nc.dram_tensor("convw", [128, 2], cdt, kind="ExternalInput")
    bias128 = nc.dram_tensor("bias128", [128, 1], F32, kind="ExternalInput")
    w1t = nc.dram_tensor("w1t", [128, 512], F32, kind="ExternalInput")
    w2t = nc.dram_tensor("w2t", [128, 256], F32, kind="ExternalInput")
    ln_g = nc.dram_tensor("ln_g", [128, 1], F32, kind="ExternalInput")
    ln_b = nc.dram_tensor("ln_b", [128, 1], F32, kind="ExternalInput")
    ident = nc.dram_tensor("ident", [128, 128], F32, kind="ExternalInput")
    logits = nc.dram_tensor("logits", [NP, NP], F32, kind="ExternalOutput")

    if exchange == 'cc':
        cc_in = nc.dram_tensor("cc_in", [ENC, NP], xdt)
        cc_out = nc.dram_tensor("cc_out", [2 * ENC, NP], xdt)

    with tile.TileContext(nc) as tc:
        with (
            tc.tile_pool(name="slab", bufs=5) as slab_pool,
            tc.tile_pool(name="x1p", bufs=2) as x1_pool,
            tc.tile_pool(name="cst", bufs=1) as cst,
            tc.tile_pool(name="work", bufs=1) as work,
            tc.tile_pool(name="rot", bufs=2) as rot,
            tc.tile_pool(name="cv", bufs=3, space="PSUM") as ps_cv,
            tc.tile_pool(name="mm", bufs=3, space="PSUM") as ps_mm,
            tc.tile_pool(name="rowt", bufs=1, space="PSUM") as ps_rt,
            tc.tile_pool(name="lg", bufs=1, space="PSUM") as ps_lg,
        ):
            # constants / weights on the scalar (ACT) queue
            convw_s = cst.tile([128, 2], cdt, tag="convw")
            bias_s = cst.tile([128, 1], F32, tag="bias")
            w1t_s = cst.tile([128, 512], F32, tag="w1t")
            w2t_s = cst.tile([128, 256], F32, tag="w2t")
            g_s = cst.tile([128, 1], F32, tag="g")
            b_s = cst.tile([128, 1], F32, tag="b")
            id_s = cst.tile([128, 128], F32, tag="id")
            ones_col = cst.tile([128, 1], F32, tag="onec")
            ones_row = cst.tile([1, 128], F32, tag="oner")
            for t, srct in ((convw_s, convw), (bias_s, bias128), (w1t_s, w1t),
                            (w2t_s, w2t), (g_s, ln_g), (b_s, ln_b),
                            (id_s, ident)):
                nc.scalar.dma_start(t[:], srct[:])
            nc.gpsimd.memset(ones_col[:], 1.0)
            nc.gpsimd.memset(ones_row[:], 1.0)

            # warm the ACT function tables (Relu/Sqrt) before the stream needs them
            scr = work.tile([1, 2], F32, tag="scr")
            nc.gpsimd.memset(scr[:], 1.0)
            nc.scalar.activation(scr[0:1, 0:1], scr[0:1, 1:2], AF.Sqrt)
            nc.scalar.activation(scr[0:1, 0:1], scr[0:1, 1:2], AF.Relu)

            # persistent tiles
            xT = [work.tile([128, NP], F32, tag=f"xT{v}", name=f"xT{v}")
                  for v in range(2)]
            hT = [work.tile([128, NP], F32, tag=f"hT{t}", name=f"hT{t}")
                  for t in range(2)]
            yT = work.tile([128, NP], F32, tag="yT")
            eT = work.tile([128, NP], xdt, tag="eT")
            B_s = work.tile([128, NP], xdt, tag="B")

            # all slab DMAs up front, alternating sync/gpsimd queues
            slabs = []
            for s in range(N_SLABS):
                st = slab_pool.tile([128, 2, CPS, NH * CPS], cdt, tag="slab")
                eng = nc.sync if s % 2 == 0 else nc.gpsimd
                eng.dma_start(st[:], feat[:, :, s * CPS:(s + 1) * CPS, :].rearrange(
                    "u p h w -> p u h w"))
                slabs.append(st)

            x1s = {}

            def conv_slab(s):
                """conv1x1 + ReLU for slab s into its chain's x1 staging."""
                k, base = SLAB_CHAIN[s]
                n_ch = CHAINS[k][2]
                if base == 0:
                    x1s[k] = x1_pool.tile([1, n_ch, PIX], F32, tag="x1",
                                          name=f"x1_{k}")
                x1 = x1s[k]
                stv = [slabs[s][:, u, :, :].rearrange("p h (c j) -> p c h j", c=NH)
                       for u in range(2)]
                for g in range(SLAB_NP // 2):
                    cvt = ps_cv.tile([1, 2 * PIX], F32, tag="cv")
                    for u in range(2):
                        nc.tensor.matmul(
                            cvt[0:1, :],
                            convw_s[:, u:u + 1],
                            stv[u][:, 2 * g:2 * g + 2, :, :],
                            start=(u == 0), stop=(u == 1),
                        )
                    # evacuate + bias + relu, alternating engines
                    o = x1[0:1, base + 2 * g:base + 2 * g + 2, :]
                    if g % 2 == 0:
                        nc.scalar.activation(o, cvt[0:1, :], AF.Relu,
                                             bias=bias_s[0:1, :])
                    else:
                        nc.vector.tensor_scalar(o, cvt[0:1, :], bias_s[0:1, :],
                                                0.0, ALU.add, ALU.max)

            chst = {}

            x_cmps = {}

            def cmp_chain(k, eng=None):
                """compact chain k's x1 staging into partition-major x_cmp."""
                n = CHAINS[k][2]
                x_cmp = rot.tile([CH_NP, PIX], F32, tag="xc", name=f"xc_{k}")
                x_cmp = x_cmp[0:n, :]
                (eng or nc.scalar).dma_start(x_cmp[:, :], x1s[k][0:1, :, :])
                x_cmps[k] = x_cmp

            def chain_a(k, tail=False, row=None, roff=0, skip_transpose=False):
                """chain k: transpose + MLP into yT[:, J] + LN stat matmuls."""
                _, p0, n = CHAINS[k]
                J = slice(p0, p0 + n)
                if not skip_transpose:
                    if k not in x_cmps:
                        cmp_chain(k)
                    x_cmp = x_cmps.pop(k)
                    for v in range(2):
                        tp = ps_mm.tile([128, n], F32, tag="mm", name=f"tp{v}_{k}")
                        nc.tensor.transpose(tp[:], x_cmp[:, 128 * v:128 * (v + 1)],
                                            id_s[0:n, 0:n])
                        if v == 0 and not tail:
                            nc.scalar.copy(xT[v][:, J], tp[:])
                        else:
                            nc.vector.tensor_copy(xT[v][:, J], tp[:])
                for t in range(2):
                    ph = ps_mm.tile([128, n], F32, tag="mm", name=f"ph{t}_{k}")
                    for v in range(2):
                        nc.tensor.matmul(
                            ph[:], w1t_s[:, 256 * v + 128 * t:256 * v + 128 * t + 128],
                            xT[v][:, J], start=(v == 0), stop=(v == 1),
                        )
                    if tail and t == 1:
                        nc.vector.tensor_scalar_max(hT[t][:, J], ph[:], 0.0)
                    else:
                        nc.scalar.activation(hT[t][:, J], ph[:], AF.Relu)
                py = ps_mm.tile([128, n], F32, tag="mm", name=f"py_{k}")
                for t in range(2):
                    nc.tensor.matmul(py[:], w2t_s[:, 128 * t:128 * t + 128],
                                     hT[t][:, J], start=(t == 0), stop=(t == 1))
                if tail:
                    nc.scalar.copy(yT[:, J], py[:])
                else:
                    nc.vector.tensor_copy(yT[:, J], py[:])
                # stats: sum(y) and sum(y^2) via ones-matmuls into one bank
                sq = rot.tile([128, n], F32, tag="sq", name=f"sq_{k}")
                nc.vector.tensor_tensor(sq[:], yT[:, J], yT[:, J], ALU.mult)
                if row is None:
                    row = ps_mm.tile([1, 64], F32, tag="mm", name=f"row_{k}")
                    chst[k] = row
                nc.tensor.matmul(row[0:1, roff:roff + n], ones_col[:], yT[:, J],
                                 start=True, stop=True)
                nc.tensor.matmul(row[0:1, 32 + roff:32 + roff + n], ones_col[:],
                                 sq[:], start=True, stop=True)

            def chain_b(k, row=None, span=None):
                """chain k: LN row math + broadcast + affine into eT[:, J]."""
                if span is not None:
                    p0, n = span
                else:
                    _, p0, n = CHAINS[k]
                J = slice(p0, p0 + n)
                if row is None:
                    row = chst.pop(k)
                rw = rot.tile([1, 128], F32, tag="rw", name=f"rw_{k}")
                mrow, qrow = rw[0:1, 0:n], rw[0:1, 20:20 + n]
                t1, veps = rw[0:1, 40:40 + n], rw[0:1, 60:60 + n]
                srow = rw[0:1, 80:80 + n]
                nc.vector.tensor_scalar_mul(mrow, row[0:1, 0:n], 1.0 / ENC)
                nc.vector.tensor_scalar_mul(qrow, row[0:1, 32:32 + n], 1.0 / ENC)
                nc.vector.tensor_tensor(t1, mrow, mrow, ALU.mult)
                nc.vector.tensor_tensor(veps, qrow, t1, ALU.subtract)
                nc.vector.tensor_scalar_add(veps, veps, LN_EPS)
                nc.scalar.activation(srow, veps, AF.Sqrt)
                # rstd into qrow slot; nmr = -mean*rstd into mrow slot
                rstd = qrow
                nc.vector.reciprocal(rstd, srow)
                # one Newton polish step: r' = r * (1.5 - 0.5*v*r^2)
                nc.vector.tensor_tensor(t1, rstd, rstd, ALU.mult)
                nc.vector.tensor_tensor(t1, t1, veps, ALU.mult)
                nc.vector.tensor_scalar(t1, t1, -0.5, 1.5, ALU.mult, ALU.add)
                nc.vector.tensor_tensor(rstd, rstd, t1, ALU.mult)
                nc.vector.tensor_tensor(mrow, mrow, rstd, ALU.mult)
                nc.vector.tensor_scalar_mul(mrow, mrow, -1.0)  # nmr
                bc = ps_mm.tile([128, 64], F32, tag="mm", name=f"bc_{k}")
                nc.tensor.matmul(bc[:, 0:n], ones_row[:], rstd, start=True, stop=True)
                nc.tensor.matmul(bc[:, 32:32 + n], ones_row[:], mrow,
                                 start=True, stop=True)
                nc.vector.tensor_tensor(yT[:, J], yT[:, J], bc[:, 0:n], ALU.mult)
                nc.vector.tensor_tensor(yT[:, J], yT[:, J], bc[:, 32:32 + n], ALU.add)
                # affine (ln_g/ln_b pre-scaled by sqrt(exp(logit_scale)) on host)
                nc.vector.tensor_scalar(eT[:, J], yT[:, J], g_s[:], b_s[:],
                                        ALU.mult, ALU.add)

            # interleave: convs stream in, chains lag by one slab to avoid
            # stalling the PE FIFO on vector-engine dependencies
            conv_slab(0); conv_slab(1); conv_slab(2)
            chain_a(0)
            conv_slab(3)
            chain_b(0)
            conv_slab(4)
            chain_a(1)
            conv_slab(5)
            chain_b(1)
            conv_slab(6)
            chain_a(2)
            conv_slab(7)
            chain_b(2)
            chain_a(3)
            conv_slab(8)
            chain_b(3)
            conv_slab(9)
            cmp_chain(4, nc.sync)      # SP queue is idle after the last slab
            row_t = ps_rt.tile([1, 64], F32, tag="rowt")
            chain_a(4, tail=True, row=row_t, roff=0)
            cmp_chain(5, nc.sync)
            chain_a(5, tail=True, row=row_t, roff=SLAB_NP)
            chain_b(5, row=row_t, span=(80, 20))

            # exchange eT within pairs
            if exchange == 'rdma':
                rsem = nc.alloc_semaphore("rdma_rsem")
                lsem = nc.alloc_semaphore("rdma_lsem")
                nc.gpsimd.remote_dma_broadcast(
                    B_s[:], eT[:], remote_sem=rsem, local_sem=lsem,
                    rdests=[(0, 1)] + [None] * 7)
                nc.gpsimd.trigger_dma(count=None)
                with tc.tile_critical():
                    nc.vector.wait_ge(rsem, 2)
                    nc.vector.tensor_copy(B_s[:], B_s[:])
            elif exchange == 'cc':
                # stage settled columns early so only the last chain's 20
                # columns pay staging latency ahead of the collective
                nc.sync.dma_start(cc_in[:, 0:80], eT[:, 0:80])
                nc.sync.dma_start(cc_in[:, 80:NP], eT[:, 80:NP])
                groups = [[2 * i, 2 * i + 1] for i in range(n_cores // 2)]
                nc.gpsimd.collective_compute(
                    "AllGather", ALU.bypass, replica_groups=groups,
                    ins=[cc_in.ap().opt()], outs=[cc_out.ap().opt()],
                )
                # even cores read the odd partner's shard (only even cores'
                # logits are collected on the host); split across the two
                # idle HWDGE queues so the loads complete in parallel
                nc.sync.dma_start(B_s[:, 0:50], cc_out[ENC:2 * ENC, 0:50])
                nc.scalar.dma_start(B_s[:, 50:NP], cc_out[ENC:2 * ENC, 50:NP])
            else:
                nc.vector.tensor_copy(B_s[:], eT[:])

            # logits = eT.T @ B  (logit scale folded into eT/B via ln_g/ln_b)
            pL = ps_lg.tile([NP, NP], F32, tag="lg")
            nc.tensor.matmul(pL[:], eT[:], B_s[:], start=True, stop=True)
            L_s = work.tile([NP, NP], F32, tag="Ls")
            nc.vector.tensor_copy(L_s[:], pL[:])
            nc.sync.dma_start(logits[0:50, :], L_s[0:50, :])
            nc.scalar.dma_start(logits[50:NP, :], L_s[50:NP, :])

    nc.compile()
    return nc


def host_inputs_for_core(core, inputs):
    """Build the per-core in_map from the full problem inputs dict."""
    b, m = core // 2, core % 2
    feat = np.asarray(inputs["feat_c1" if m == 0 else "feat_c2"])[b]
    pre = "img_" if m == 0 else "depth_"
    s_half = np.sqrt(np.exp(np.float32(np.asarray(inputs["logit_scale"]))))
    cw = np.asarray(inputs[pre + "conv_w"]).reshape(2, 128).T.copy()   # [128,2]
    cb = np.full((128, 1), np.asarray(inputs[pre + "conv_b"])[0], np.float32)
    w1 = np.asarray(inputs[pre + "w1"])  # [256,256] (o=128t+m', i=128v+k)
    w1t = np.ascontiguousarray(
        w1.reshape(2, 128, 2, 128).transpose(3, 2, 0, 1).reshape(128, 512))
    w2 = np.asarray(inputs[pre + "w2"])  # [128,256]
    w2t = np.ascontiguousarray(
        w2.reshape(128, 2, 128).transpose(2, 1, 0).reshape(128, 256))
    return {
        "feat": np.ascontiguousarray(feat).reshape(2, 128, 160, 160),
        "convw": cw.astype(np.float32),
        "bias128": cb,
        "w1t": w1t.astype(np.float32),
        "w2t": w2t.astype(np.float32),
        "ln_g": (np.asarray(inputs[pre + "ln_g"]) * s_half)
            .reshape(128, 1).astype(np.float32),
        "ln_b": (np.asarray(inputs[pre + "ln_b"]) * s_half)
            .reshape(128, 1).astype(np.float32),
        "ident": np.eye(128, dtype=np.float32),
    }


_NC_CACHE = {}


def _get_nc():
    if "nc" not in _NC_CACHE:
        import os
        nc = bacc.Bacc("TRN2", target_bir_lowering=False, num_devices=8)
        build_kernel(nc, exchange=os.environ.get("BASS_EXCHANGE", "cc"),
                     xdt=BF16 if os.environ.get("BASS_XDT") == "bf16" else F32)
        _NC_CACHE["nc"] = nc
    return _NC_CACHE["nc"]


def kernel(**inputs):
    nc = _get_nc()
    in_maps = [host_inputs_for_core(c, inputs) for c in range(8)]
    res = run_bass_kernel_spmd(nc, in_maps, list(range(8)))
    logits_img = np.stack([np.asarray(res.results[2 * b]["logits"])
                           for b in range(4)]).astype(np.float32)
    logits_depth = np.ascontiguousarray(logits_img.transpose(0, 2, 1))
    return logits_img, logits_depth


# revision 52
# speedup vs baseline: 1.0431x; 1.0411x over previous
"""Trainium2 Bass kernel for nn_ContrastiveLearning (self-contained).

kernel(**inputs) takes the FULL unsharded inputs (as produced by the
problem's setup_inputs) and returns (logits_per_img, logits_per_depth),
each [4, 100, 100] fp32.

Sharding: 8 NeuronCores, core c = (batch b=c//2, modality m=c%2). Each core
streams its 26 MB feature map in 10 one-patch-row slabs (2.62 MB each,
alternating between the sync and gpsimd DMA queues), computes
conv1x1+ReLU per slab, then runs the patch MLP + LayerNorm per slab-PAIR
(5 chains of 20 patches, interleaved with the conv stream so only the last
chain sits in the tail), exchanges eT with its pair partner (2-core
AllGather), and computes the 100x100 contrastive logits on-device.
sqrt(exp(logit_scale)) is folded into the LayerNorm affine on the host, so
no logits scaling happens on device.
"""
import numpy as np
import concourse.bass as bass
import concourse.bacc as bacc
import concourse.mybir as mybir
import concourse.tile as tile
from concourse.bass_utils import run_bass_kernel_spmd


F32 = mybir.dt.float32
F32R = mybir.dt.float32r
AF = mybir.ActivationFunctionType
ALU = mybir.AluOpType

NV = NH = 10          # patch grid
NP = NV * NH          # 100 patches
CPS = 16
ENC = 128
PIX = CPS * CPS       # 256 features per patch
LN_EPS = 1e-5
N_SLABS = 10          # one patch-row (16 rows x 160 cols) per slab
SLAB_NP = NP // N_SLABS   # 10 patches per slab
CH_NP = 2 * SLAB_NP       # 20 patches per chain (slab pair)


BF16 = mybir.dt.bfloat16

# chains of 20 patches (slab pairs); slabs 8/9 ride the scalar (ACT) queue
# EARLY so the last arrivals are slabs 6/7 and chain 3 is the only tail chain
CHAINS = [(0, 0, 20), (1, 20, 20), (2, 40, 20), (3, 60, 20), (4, 80, 20)]
SLAB_CHAIN = {0: (0, 0), 1: (0, 10), 2: (1, 0), 3: (1, 10),
              4: (2, 0), 5: (2, 10), 6: (3, 0), 7: (3, 10),
              8: (4, 0), 9: (4, 10)}
# DMA issue order (defines slab-pool buffer rotation) and queue per slab
SLAB_ORDER = [8, 9, 0, 1, 2, 3, 4, 5, 6, 7]
SLAB_QUEUE = {0: 'sync', 2: 'sync', 4: 'sync', 6: 'sync',
              1: 'gpsimd', 3: 'gpsimd', 5: 'gpsimd', 7: 'gpsimd',
              8: 'scalar', 9: 'scalar'}
# slabs whose conv evacuation must avoid ACT (it is DMA-holding early)
EVAC_DVE_ONLY = {0, 1, 8}


# NOTE: bf16 exchange payload (xdt=BF16) corrupts data through the real
# AllGather on this stack — keep the exchange in f32
def build_kernel(nc, exchange='cc', n_cores=8, conv_f32r=True, xdt=F32):
    cdt = F32R if conv_f32r else F32

    feat = nc.dram_tensor("feat", [2, 128, 160, 160], cdt, kind="ExternalInput")
    convw = nc.dram_tensor("convw", [128, 2], cdt, kind="ExternalInput")
    # w1t | w2t | ident | bias | ln_g | ln_b packed into one DMA
    wpack = nc.dram_tensor("wpack", [128, 899], F32, kind="ExternalInput")
    logits = nc.dram_tensor("logits", [NP, NP], F32, kind="ExternalOutput")

    if exchange == 'cc':
        cc_in = nc.dram_tensor("cc_in", [ENC, NP], xdt)
        cc_out = nc.dram_tensor("cc_out", [2 * ENC, NP], xdt)

    with tile.TileContext(nc) as tc:
        with (
            tc.tile_pool(name="slab", bufs=7) as slab_pool,
            tc.tile_pool(name="x1p", bufs=2) as x1_pool,
            tc.tile_pool(name="cst", bufs=1) as cst,
            tc.tile_pool(name="work", bufs=1) as work,
            tc.tile_pool(name="rot", bufs=2) as rot,
            tc.tile_pool(name="cv", bufs=3, space="PSUM") as ps_cv,
            tc.tile_pool(name="mm", bufs=4, space="PSUM") as ps_mm,
            tc.tile_pool(name="lg", bufs=1, space="PSUM") as ps_lg,
        ):
            # constants / weights: two DMAs on the scalar (ACT) queue, issued
            # before its slab DMAs so conv weights land by the first conv
            convw_s = cst.tile([128, 2], cdt, tag="convw")
            wp_s = cst.tile([128, 899], F32, tag="wpack")
            nc.scalar.dma_start(convw_s[:], convw[:])
            nc.scalar.dma_start(wp_s[:], wpack[:])
            w1t_s = wp_s[:, 0:512]
            w2t_s = wp_s[:, 512:768]
            id_s = wp_s[:, 768:896]
            bias_s = wp_s[:, 896:897]
            g_s = wp_s[:, 897:898]
            b_s = wp_s[:, 898:899]
            ones_col = cst.tile([128, 1], F32, tag="onec")
            ones_row = cst.tile([1, 128], F32, tag="oner")

            # persistent tiles
            xT = [work.tile([128, NP], F32, tag=f"xT{v}", name=f"xT{v}")
                  for v in range(2)]
            hT = [work.tile([128, NP], F32, tag=f"hT{t}", name=f"hT{t}")
                  for t in range(2)]
            yT = work.tile([128, NP], F32, tag="yT")
            eT = work.tile([128, NP], xdt, tag="eT")
            B_s = work.tile([128, NP], xdt, tag="B")

            # all slab DMAs up front across three queues (sync / gpsimd /
            # scalar); slabs 8-9 ride scalar right after the weights, so the
            # last arrivals are slabs 6/7. Allocation order (SLAB_ORDER)
            # controls buffer rotation; issue order controls sem lanes.
            slabs = {}
            for s in SLAB_ORDER:
                slabs[s] = slab_pool.tile([128, 2, CPS, NH * CPS], cdt,
                                          tag="slab", name=f"st{s}")
            nc.gpsimd.memset(ones_col[:], 1.0)
            nc.gpsimd.memset(ones_row[:], 1.0)
            for s in [8, 9, 0, 1, 2, 3, 4, 5, 6, 7]:
                eng = getattr(nc, SLAB_QUEUE[s])
                eng.dma_start(slabs[s][:],
                              feat[:, :, s * CPS:(s + 1) * CPS, :].rearrange(
                                  "u p h w -> p u h w"))

            x1s = {}

            def conv_slab(s):
                """conv1x1 + ReLU for slab s into its chain's x1 staging."""
                k, base = SLAB_CHAIN[s]
                n_ch = CHAINS[k][2]
                if base == 0:
                    x1s[k] = x1_pool.tile([1, n_ch, PIX], F32, tag="x1",
                                          name=f"x1_{k}")
                x1 = x1s[k]
                stv = [slabs[s][:, u, :, :].rearrange("p h (c j) -> p c h j", c=NH)
                       for u in range(2)]
                for g in range(SLAB_NP // 2):
                    cvt = ps_cv.tile([1, 2 * PIX], F32, tag="cv")
                    for u in range(2):
                        nc.tensor.matmul(
                            cvt[0:1, :],
                            convw_s[:, u:u + 1],
                            stv[u][:, 2 * g:2 * g + 2, :, :],
                            start=(u == 0), stop=(u == 1),
                        )
                    # evacuate + bias + relu, alternating engines (all-DVE
                    # while the scalar queue is still streaming slabs 8-9)
                    o = x1[0:1, base + 2 * g:base + 2 * g + 2, :]
                    if g % 2 == 0 and s not in EVAC_DVE_ONLY:
                        nc.scalar.activation(o, cvt[0:1, :], AF.Relu,
                                             bias=bias_s[0:1, :])
                    else:
                        nc.vector.tensor_scalar(o, cvt[0:1, :], bias_s[0:1, :],
                                                0.0, ALU.add, ALU.max)

            chst = {}

            x_cmps = {}

            def cmp_chain(k, eng=None):
                """compact chain k's x1 staging into partition-major x_cmp."""
                n = CHAINS[k][2]
                x_cmp = rot.tile([CH_NP, PIX], F32, tag="xc", name=f"xc_{k}")
                x_cmp = x_cmp[0:n, :]
                (eng or nc.scalar).dma_start(x_cmp[:, :], x1s[k][0:1, :, :])
                x_cmps[k] = x_cmp

            def chain_a(k, tail=False, row=None, roff=0, skip_transpose=False):
                """chain k: transpose + MLP into yT[:, J] + LN stat matmuls."""
                _, p0, n = CHAINS[k]
                J = slice(p0, p0 + n)
                if not skip_transpose:
                    if k not in x_cmps:
                        cmp_chain(k)
                    x_cmp = x_cmps.pop(k)
                    for v in range(2):
                        tp = ps_mm.tile([128, n], F32, tag="mm", name=f"tp{v}_{k}")
                        nc.tensor.transpose(tp[:], x_cmp[:, 128 * v:128 * (v + 1)],
                                            id_s[0:n, 0:n])
                        if v == 0 and not tail:
                            nc.scalar.copy(xT[v][:, J], tp[:])
                        else:
                            nc.vector.tensor_copy(xT[v][:, J], tp[:])
                for t in range(2):
                    ph = ps_mm.tile([128, n], F32, tag="mm", name=f"ph{t}_{k}")
                    for v in range(2):
                        nc.tensor.matmul(
                            ph[:], w1t_s[:, 256 * v + 128 * t:256 * v + 128 * t + 128],
                            xT[v][:, J], start=(v == 0), stop=(v == 1),
                        )
                    if tail and t == 1:
                        nc.vector.tensor_scalar_max(hT[t][:, J], ph[:], 0.0)
                    else:
                        nc.scalar.activation(hT[t][:, J], ph[:], AF.Relu)
                py = ps_mm.tile([128, n], F32, tag="mm", name=f"py_{k}")
                for t in range(2):
                    nc.tensor.matmul(py[:], w2t_s[:, 128 * t:128 * t + 128],
                                     hT[t][:, J], start=(t == 0), stop=(t == 1))
                if tail:
                    nc.scalar.copy(yT[:, J], py[:])
                else:
                    nc.vector.tensor_copy(yT[:, J], py[:])
                # stats: sum(y) and sum(y^2) via ones-matmuls into one bank
                sq = rot.tile([128, n], F32, tag="sq", name=f"sq_{k}")
                nc.vector.tensor_tensor(sq[:], yT[:, J], yT[:, J], ALU.mult)
                if row is None:
                    row = ps_mm.tile([1, 64], F32, tag="mm", name=f"row_{k}")
                    chst[k] = row
                nc.tensor.matmul(row[0:1, roff:roff + n], ones_col[:], yT[:, J],
                                 start=True, stop=True)
                nc.tensor.matmul(row[0:1, 32 + roff:32 + roff + n], ones_col[:],
                                 sq[:], start=True, stop=True)

            def chain_b(k, row=None, span=None):
                """chain k: LN row math + broadcast + affine into eT[:, J]."""
                if span is not None:
                    p0, n = span
                else:
                    _, p0, n = CHAINS[k]
                J = slice(p0, p0 + n)
                if row is None:
                    row = chst.pop(k)
                rw = rot.tile([1, 128], F32, tag="rw", name=f"rw_{k}")
                mrow, qrow = rw[0:1, 0:n], rw[0:1, 20:20 + n]
                t1, veps = rw[0:1, 40:40 + n], rw[0:1, 60:60 + n]
                srow = rw[0:1, 80:80 + n]
                nc.vector.tensor_scalar_mul(mrow, row[0:1, 0:n], 1.0 / ENC)
                nc.vector.tensor_scalar_mul(qrow, row[0:1, 32:32 + n], 1.0 / ENC)
                nc.vector.tensor_tensor(t1, mrow, mrow, ALU.mult)
                nc.vector.tensor_tensor(veps, qrow, t1, ALU.subtract)
                nc.vector.tensor_scalar_add(veps, veps, LN_EPS)
                nc.scalar.activation(srow, veps, AF.Sqrt)
                # rstd into qrow slot; nmr = -mean*rstd into mrow slot
                rstd = qrow
                nc.vector.reciprocal(rstd, srow)
                # one Newton polish step: r' = r * (1.5 - 0.5*v*r^2)
                nc.vector.tensor_tensor(t1, rstd, rstd, ALU.mult)
                nc.vector.tensor_tensor(t1, t1, veps, ALU.mult)
                nc.vector.tensor_scalar(t1, t1, -0.5, 1.5, ALU.mult, ALU.add)
                nc.vector.tensor_tensor(rstd, rstd, t1, ALU.mult)
                nc.vector.tensor_tensor(mrow, mrow, rstd, ALU.mult)
                nc.vector.tensor_scalar_mul(mrow, mrow, -1.0)  # nmr
                bc = ps_mm.tile([128, 64], F32, tag="mm", name=f"bc_{k}")
                nc.tensor.matmul(bc[:, 0:n], ones_row[:], rstd, start=True, stop=True)
                nc.tensor.matmul(bc[:, 32:32 + n], ones_row[:], mrow,
                                 start=True, stop=True)
                nc.vector.tensor_tensor(yT[:, J], yT[:, J], bc[:, 0:n], ALU.mult)
                nc.vector.tensor_tensor(yT[:, J], yT[:, J], bc[:, 32:32 + n], ALU.add)
                # affine (ln_g/ln_b pre-scaled by sqrt(exp(logit_scale)) on host)
                nc.vector.tensor_scalar(eT[:, J], yT[:, J], g_s[:], b_s[:],
                                        ALU.mult, ALU.add)

            # process in ARRIVAL order (s8/s9 stream early on the scalar
            # queue); chains lag their data by ~one slab so PE never stalls
            # on vector-engine dependencies; chain 3 (slabs 6/7, the last
            # arrivals) is the only chain left in the tail
            conv_slab(0); conv_slab(1)
            conv_slab(8)
            chain_a(0)
            conv_slab(2)
            chain_b(0)
            conv_slab(3)
            conv_slab(9)
            chain_a(1)
            chain_b(1)
            conv_slab(4)
            chain_a(4)
            conv_slab(5)
            chain_b(4)
            chain_a(2)                 # slabs 4/5 settled; runs before s6/s7 land
            chain_b(2)
            conv_slab(6)
            conv_slab(7)
            cmp_chain(3, nc.sync)      # SP queue is idle after slab 6
            chain_a(3, tail=True)
            chain_b(3)

            # exchange eT within pairs
            if exchange == 'rdma':
                rsem = nc.alloc_semaphore("rdma_rsem")
                lsem = nc.alloc_semaphore("rdma_lsem")
                nc.gpsimd.remote_dma_broadcast(
                    B_s[:], eT[:], remote_sem=rsem, local_sem=lsem,
                    rdests=[(0, 1)] + [None] * 7)
                nc.gpsimd.trigger_dma(count=None)
                with tc.tile_critical():
                    nc.vector.wait_ge(rsem, 2)
                    nc.vector.tensor_copy(B_s[:], B_s[:])
            elif exchange == 'cc':
                # stage settled columns early so only the last chain's 20
                # columns pay staging latency ahead of the collective
                nc.sync.dma_start(cc_in[:, 0:80], eT[:, 0:80])
                nc.sync.dma_start(cc_in[:, 80:NP], eT[:, 80:NP])
                groups = [[2 * i, 2 * i + 1] for i in range(n_cores // 2)]
                nc.gpsimd.collective_compute(
                    "AllGather", ALU.bypass, replica_groups=groups,
                    ins=[cc_in.ap().opt()], outs=[cc_out.ap().opt()],
                )
                # even cores read the odd partner's shard (only even cores'
                # logits are collected on the host); split across the two
                # idle HWDGE queues so the loads complete in parallel
                nc.sync.dma_start(B_s[:, 0:50], cc_out[ENC:2 * ENC, 0:50])
                nc.scalar.dma_start(B_s[:, 50:NP], cc_out[ENC:2 * ENC, 50:NP])
            else:
                nc.vector.tensor_copy(B_s[:], eT[:])

            # logits = eT.T @ B  (logit scale folded into eT/B via ln_g/ln_b)
            pL = ps_lg.tile([NP, NP], F32, tag="lg")
            nc.tensor.matmul(pL[:], eT[:], B_s[:], start=True, stop=True)
            L_s = work.tile([NP, NP], F32, tag="Ls")
            nc.vector.tensor_copy(L_s[:], pL[:])
            nc.sync.dma_start(logits[0:50, :], L_s[0:50, :])
            nc.scalar.dma_start(logits[50:NP, :], L_s[50:NP, :])

    nc.compile()
    return nc


def host_inputs_for_core(core, inputs):
    """Build the per-core in_map from the full problem inputs dict."""
    b, m = core // 2, core % 2
    feat = np.asarray(inputs["feat_c1" if m == 0 else "feat_c2"])[b]
    pre = "img_" if m == 0 else "depth_"
    s_half = np.sqrt(np.exp(np.float32(np.asarray(inputs["logit_scale"]))))
    cw = np.asarray(inputs[pre + "conv_w"]).reshape(2, 128).T.copy()   # [128,2]
    cb = np.full((128, 1), np.asarray(inputs[pre + "conv_b"])[0], np.float32)
    w1 = np.asarray(inputs[pre + "w1"])  # [256,256] (o=128t+m', i=128v+k)
    w1t = np.ascontiguousarray(
        w1.reshape(2, 128, 2, 128).transpose(3, 2, 0, 1).reshape(128, 512))
    w2 = np.asarray(inputs[pre + "w2"])  # [128,256]
    w2t = np.ascontiguousarray(
        w2.reshape(128, 2, 128).transpose(2, 1, 0).reshape(128, 256))
    wpack = np.concatenate([
        w1t.astype(np.float32),
        w2t.astype(np.float32),
        np.eye(128, dtype=np.float32),
        cb,
        (np.asarray(inputs[pre + "ln_g"]) * s_half).reshape(128, 1)
            .astype(np.float32),
        (np.asarray(inputs[pre + "ln_b"]) * s_half).reshape(128, 1)
            .astype(np.float32),
    ], axis=1)
    return {
        "feat": np.ascontiguousarray(feat).reshape(2, 128, 160, 160),
        "convw": cw.astype(np.float32),
        "wpack": np.ascontiguousarray(wpack),
    }


_NC_CACHE = {}


def _get_nc():
    if "nc" not in _NC_CACHE:
        import os
        nc = bacc.Bacc("TRN2", target_bir_lowering=False, num_devices=8)
        build_kernel(nc, exchange=os.environ.get("BASS_EXCHANGE", "cc"),
                     xdt=BF16 if os.environ.get("BASS_XDT") == "bf16" else F32)
        _NC_CACHE["nc"] = nc
    return _NC_CACHE["nc"]


def kernel(**inputs):
    nc = _get_nc()
    in_maps = [host_inputs_for_core(c, inputs) for c in range(8)]
    res = run_bass_kernel_spmd(nc, in_maps, list(range(8)))
    logits_img = np.stack([np.asarray(res.results[2 * b]["logits"])
                           for b in range(4)]).astype(np.float32)
    logits_depth = np.ascontiguousarray(logits_img.transpose(0, 2, 1))
    return logits_img, logits_depth
